# revision 24
# baseline (speedup 1.0000x reference)
"""Causal self-attention Trainium2 Bass kernel (fp16 SBUF-resident rewrite).

Problem: B=4, T=2048, C=2048, H=16 heads, D=128 head dim, fp32 in/out.
  qkv = x @ w_qkv ; causal softmax(q k^T / sqrt(D)) v ; out = av @ w_proj

Sharding (8 NeuronCores): DP=4 over batch x TP=2 over head groups
(Megatron-style: w_qkv columns / w_proj rows split by heads). Core
c handles batch b=c//2, heads g*8..g*8+8 with g=c%2. Each core emits a
partial [T, C] f32 output; host sums the TP pairs.

Key differences vs the fp32r/DRAM-scratch version:
  * All operands stored fp16 (inputs cast host-side); matmuls run fp16
    at the same 1 cycle/row PE rate as f32r but with half the SBUF/DMA
    traffic, so q/k/v/av stay SBUF-resident end to end (no DRAM round
    trip between phases).
  * Softmax denominators: exp tiles accumulate on DVE (fp16 4x mode,
    lagged behind the exps so they never gate them); the partition
    reduction is one tiny [128,1]-stationary PE matmul per (head, si)
    and the broadcast of 1/den rides the otherwise-idle gpsimd.
  * Exp runs in [128,1024] two-PSUM-bank mega tiles (halves the ACT
    per-instruction access overhead); a diagonal pair left-aligns its
    second block so the pair is one contiguous exp with no garbage.
  * Phase 2 is one globally software-pipelined stream of block-pairs
    (scores two pairs ahead of AV); short diagonal-heavy si groups are
    woven into the long clean ones so the pipeline never runs shallow.
  * Phase-3 partials are written f16 (summed in f32 on host) and the
    wp loads are drip-fed on the pool queue mid-phase-2 because DMAs
    in flight across a pool transition stall its semaphore rundown.
"""

import math
import os
import sys
from collections import deque

import numpy as np

for _p in ("/opt/trn_rl_repo",):
    if _p not in sys.path:
        sys.path.insert(0, _p)

import concourse.mybir as mybir
from concourse import bacc
from concourse.tile import TileContext

B, T, C, H, D = 4, 2048, 2048, 16, 128
P = 128
NCORES = 8
HL = 8          # heads per core (local)
FL = HL * D     # local feature dim = 1024
NCC = C // P    # 16 contraction chunks
NTB = T // 512  # 4 query superblocks
NTC = T // P    # 16 t chunks
EXP_SCALE = 1.0 / math.sqrt(D)
NEG = -1.0e30

f32 = mybir.dt.float32
f16 = mybir.dt.float16


def build_nc():
    nc = bacc.Bacc()
    xt_d = nc.declare_dram_parameter("xt", [C, T], f16, isOutput=False)
    # wqk packed host-side as [jc, p, cc, col]: jc 0..7 = q head jc,
    # jc 8..15 = k head jc-8; one contiguous [128, 16*128] DMA per jc.
    wqk_d = nc.declare_dram_parameter("wqk", [16, P, NCC, P], f16, isOutput=False)
    wv_d = nc.declare_dram_parameter("wv", [NCC, P, FL], f16, isOutput=False)
    wp_d = nc.declare_dram_parameter("wp", [FL, C], f16, isOutput=False)
    masks_d = nc.declare_dram_parameter("masks", [P, P], f32, isOutput=False)
    ones_d = nc.declare_dram_parameter("ones", [P, P], f16, isOutput=False)
    out_d = nc.declare_dram_parameter("out", [T, C], f16, isOutput=True)

    ACT = mybir.ActivationFunctionType

    with TileContext(nc) as tc:
        with tc.tile_pool(name="const", bufs=1) as const_pool, \
             tc.tile_pool(name="qkt", bufs=1) as qkt_pool, \
             tc.tile_pool(name="vv", bufs=1) as v_pool:
            mask_sb = const_pool.tile([P, P], f32)
            nc.gpsimd.dma_start(mask_sb[:], masks_d[:])
            ones_sb = const_pool.tile([P, P], f16)
            nc.gpsimd.dma_start(ones_sb[:], ones_d[:])

            qkts = [qkt_pool.tile([P, T], f16, tag=f"qkt{j}", name=f"qkt{j}")
                    for j in range(16)]
            v_all = v_pool.tile([P, NTC, FL], f16, tag="vall")

            # ---------------- Phase 1: QKV projection ----------------
            with tc.tile_pool(name="xtp", bufs=1) as xt_pool:
                xts = [xt_pool.tile([P, T], f16, tag=f"xt{cc}", name=f"xt{cc}")
                       for cc in range(NCC)]

                with tc.tile_pool(name="ps1", bufs=8, space="PSUM") as ps1, \
                     tc.tile_pool(name="wv1", bufs=1) as wv_pool:
                    wvts = []
                    with tc.tile_pool(name="w1", bufs=3) as w_pool:
                        # q0/k0 weight chunks first so head 0's attention
                        # inputs exist as early as possible (jc 0 = q head 0,
                        # jc 8 = k head 0).
                        w01 = []
                        for jc in (0, 8):
                            wt_ = w_pool.tile([P, NCC, P], f16, tag="wqk",
                                              name=f"wqk{jc}")
                            nc.scalar.dma_start(wt_[:, 0:4, :], wqk_d[jc, :, 0:4, :])
                            nc.scalar.dma_start(wt_[:, 4:NCC, :],
                                                wqk_d[jc, :, 4:NCC, :])
                            w01.append(wt_)
                        # first chunk split in halves for a faster first
                        # matmul; remaining chunks rotate over three queues
                        nc.sync.dma_start(xts[0][:, 0:1024], xt_d[0:P, 0:1024])
                        nc.sync.dma_start(xts[0][:, 1024:T],
                                          xt_d[0:P, 1024:T])
                        for cc in range(1, NCC):
                            eng = nc.sync if cc % 2 == 0 else nc.scalar
                            eng.dma_start(
                                xts[cc][:], xt_d[cc * P:(cc + 1) * P, :])

                        # jc 0/8 emitted cc-outer: the PE's in-order queue
                        # then tracks xt chunk arrivals instead of blocking
                        # on the full xt load.
                        ps01 = [ps1.tile([P, 512], f32, tag="ps1",
                                         name=f"ps01_{g}") for g in range(8)]
                        # PSUM accumulation is commutative: consume chunks in
                        # expected DMA-arrival order (sync evens lead, scalar
                        # odds trail the wqk loads) so the PE never starves
                        cc_order = ([0] + list(range(2, NCC, 2))
                                    + list(range(1, NCC, 2)))
                        for ci, cc in enumerate(cc_order):
                            for wi in range(2):
                                for tb in range(NTB):
                                    nc.tensor.matmul(
                                        ps01[wi * NTB + tb][:],
                                        w01[wi][:, cc, :],
                                        xts[cc][:, tb * 512:(tb + 1) * 512],
                                        start=(ci == 0), stop=(ci == NCC - 1))
                        for wi, jc in enumerate((0, 8)):
                            for tb in range(NTB):
                                nc.scalar.copy(
                                    out=qkts[jc][:, tb * 512:(tb + 1) * 512],
                                    in_=ps01[wi * NTB + tb][:])

                        for jc in [j for j in range(16) if j not in (0, 8)]:
                            wt_ = w_pool.tile([P, NCC, P], f16, tag="wqk")
                            nc.scalar.dma_start(wt_[:], wqk_d[jc])
                            # cc-outer / tb-inner: 4 consecutive matmuls share
                            # the same stationary tile
                            pss = [ps1.tile([P, 512], f32, tag="ps1",
                                            name=f"psj{jc}_{tb}")
                                   for tb in range(NTB)]
                            for cc in range(NCC):
                                for tb in range(NTB):
                                    nc.tensor.matmul(
                                        pss[tb][:], wt_[:, cc, :],
                                        xts[cc][:, tb * 512:(tb + 1) * 512],
                                        start=(cc == 0), stop=(cc == NCC - 1))
                            for tb in range(NTB):
                                nc.scalar.copy(
                                    out=qkts[jc][:, tb * 512:(tb + 1) * 512],
                                    in_=pss[tb][:])

                        # wv loads stream behind the wqk stream during P1a so
                        # phase 1b starts without a handoff stall
                        for cc in range(NCC):
                            wt_ = wv_pool.tile([P, FL], f16, tag=f"wv{cc}",
                                               name=f"wv{cc}")
                            nc.scalar.dma_start(wt_[:], wv_d[cc])
                            wvts.append(wt_)

                    # 1b: v in natural [t, d] layout
                    if True:
                        for vb in range(FL // 512):
                            for tch in range(NTC):
                                ps = ps1.tile([P, 512], f32, tag="ps1")
                                for cc in range(NCC):
                                    nc.tensor.matmul(
                                        ps[:],
                                        xts[cc][:, tch * P:(tch + 1) * P],
                                        wvts[cc][:, vb * 512:(vb + 1) * 512],
                                        start=(cc == 0), stop=(cc == NCC - 1))
                                nc.scalar.copy(
                                    out=v_all[:, tch, vb * 512:(vb + 1) * 512],
                                    in_=ps[:])

            # ---------------- Phase 2: attention ----------------
            with tc.tile_pool(name="avt", bufs=1) as avt_pool, \
                 tc.tile_pool(name="wpp", bufs=1) as wp_pool:
                avts = [avt_pool.tile([P, T], f16, tag=f"avt{h}", name=f"avt{h}")
                        for h in range(HL)]
                wps = [wp_pool.tile([P, C], f16, tag=f"wp{f}", name=f"wp{f}")
                       for f in range(HL)]

                with tc.tile_pool(name="p2sb", bufs=1) as p2sb, \
                     tc.tile_pool(name="ps2", bufs=1, space="PSUM") as ps2:

                    def si_group(h, si, qt, kt):
                        njc = 4 * si + 4
                        av_ps = ps2.tile([P, 512], f32, tag="av",
                                         bufs=2, name=f"av_{h}_{si}")
                        sumacc = p2sb.tile([P, 512], f16, tag="sum",
                                           bufs=4)
                        return [(h, si, pp, njc, qt, kt, av_ps, sumacc)
                                for pp in range(njc // 2)]

                    def weave(a, b):
                        """Spread the short group b inside the long group a
                        so the pair pipeline never runs shallow and diagonal
                        (masked, partial-width) pairs are kept apart."""
                        out, bi = [], 0
                        na, nb = len(a), len(b)
                        for i, r in enumerate(a):
                            out.append(r)
                            while bi < nb and (bi + 1) * na <= (i + 1) * (nb + 1):
                                out.append(b[bi])
                                bi += 1
                        out.extend(b[bi:])
                        return out

                    # wp loads ride the pool DMA queue well after the phase
                    # boundary (in-flight DMAs crossing the transition stall
                    # its semaphore rundown); drained one per AV pair during
                    # head 1 so no single burst delays the broadcasts
                    wpq = [f for f in range(HL)]

                    def gen_pairs():
                        """Yield per-pair emission records across all
                        (head, si); scores run LOOK pairs ahead of AV."""
                        for h in range(HL):
                            qt, kt = qkts[h], qkts[8 + h]
                            recs = weave(si_group(h, 3, qt, kt),
                                         si_group(h, 0, qt, kt))
                            recs += weave(si_group(h, 2, qt, kt),
                                          si_group(h, 1, qt, kt))
                            yield from recs

                    def emit_sc(rec):
                        h, si, pp, njc, qt, kt, av_ps, sumacc = rec
                        if si == 0 and pp == 0:
                            nc.vector.memset(sumacc[:], 0)
                        sc = ps2.tile([P, 1024], f32, tag="sc", bufs=3)
                        et = p2sb.tile([P, 1024], f16, tag="et", bufs=6)
                        infos = []
                        for jj in (0, 1):
                            j = 2 * pp + jj
                            diag = j * P - si * 512
                            d_off = max(0, diag)
                            # left-align the second diagonal block so the
                            # pair's valid region is contiguous -> one exp
                            base = jj * 512 - (d_off if jj else 0)
                            nc.tensor.matmul(
                                sc[:, base + d_off:base + 512],
                                kt[:, j * P:(j + 1) * P],
                                qt[:, si * 512 + d_off:(si + 1) * 512],
                                start=True, stop=True)
                            # mask immediately after its score matmul so exp
                            # is never queued behind lower-priority DVE work
                            if diag >= 0:
                                nc.vector.tensor_add(
                                    out=sc[:, base + d_off:base + d_off + P],
                                    in0=sc[:, base + d_off:base + d_off + P],
                                    in1=mask_sb[:])
                            infos.append((j, diag >= 0, d_off, base))
                        lo = infos[0][2] + infos[0][3]  # d_off0 (+ base0 == 0)
                        hi = infos[1][3] + 512          # base1 + 512
                        nc.scalar.activation(et[:, lo:hi], sc[:, lo:hi],
                                             ACT.Exp, scale=EXP_SCALE)
                        return (rec, et, infos)

                    def emit_av(sc_rec):
                        (h, si, pp, njc, qt, kt, av_ps, sumacc), et, infos = sc_rec
                        last_pair = (2 * pp + 1 == njc - 1)
                        if h >= 1 and wpq:
                            f = wpq.pop(0)
                            nc.gpsimd.dma_start(
                                wps[f][:], wp_d[f * P:(f + 1) * P, :])
                        for (j, is_diag, d_off, base) in infos:
                            nc.tensor.matmul(
                                av_ps[:, d_off:],
                                v_all[:, j, h * P:(h + 1) * P],
                                et[:, base + d_off:base + 512],
                                start=(j == 0), stop=(j == njc - 1))
                        if not last_pair:
                            # denominator accumulation lags the exps by LOOK
                            # pairs: it only gates the si-end ones-matmul,
                            # never the next exp. First pair initializes
                            # sumacc without a memset (except si 0, whose
                            # first pair is diagonal/partial-width).
                            if pp == 0 and si > 0:
                                nc.vector.scalar_tensor_tensor(
                                    out=sumacc[:], in0=et[:, 0:512],
                                    scalar=1.0, in1=et[:, 512:1024],
                                    op0=mybir.AluOpType.mult,
                                    op1=mybir.AluOpType.add)
                            else:
                                for (j, is_diag, d_off, base) in infos:
                                    nc.vector.tensor_add(
                                        out=sumacc[:, d_off:],
                                        in0=sumacc[:, d_off:],
                                        in1=et[:, base + d_off:base + 512])
                        else:
                            # close out softmax. The last pair's exps are
                            # summed directly by the PE ones-matmul (PSUM
                            # accumulation), so the chain never waits on a
                            # trailing DVE sumadd; broadcast rides gpsimd
                            # off the critical path.
                            den = ps2.tile([P, 1024], f32, tag="sc", bufs=3,
                                           name=f"den_{h}_{si}")
                            nc.tensor.matmul(
                                den[0:1, 0:512], ones_sb[:, 0:1], sumacc[:],
                                start=True, stop=False)
                            for (j, is_diag, d_off, base) in infos:
                                nc.tensor.matmul(
                                    den[0:1, d_off:512], ones_sb[:, 0:1],
                                    et[:, base + d_off:base + 512],
                                    start=False, stop=(j == njc - 1))
                            rec32 = p2sb.tile([1, 512], f32, tag="rec32",
                                              bufs=3)
                            nc.vector.reciprocal_approx_fast(
                                out=rec32[:], in_=den[0:1, 0:512])
                            recb = p2sb.tile([P, 512], f32, tag="recb",
                                             bufs=3)
                            nc.gpsimd.partition_broadcast(recb[:], rec32[:])
                            nc.vector.tensor_mul(
                                out=avts[h][:, si * 512:(si + 1) * 512],
                                in0=av_ps[:], in1=recb[:])

                    LOOK = 2
                    pend = deque()
                    for rec in gen_pairs():
                        pend.append(emit_sc(rec))
                        if len(pend) > LOOK:
                            emit_av(pend.popleft())
                    while pend:
                        emit_av(pend.popleft())

                # ---------------- Phase 3: output projection ----------------
                with tc.tile_pool(name="cb3", bufs=8) as cb3, \
                     tc.tile_pool(name="ps3", bufs=8, space="PSUM") as ps3:
                    for tch in range(NTC):
                        ob = cb3.tile([P, C], f16, tag="ocb")
                        for cb_ in range(C // 512):
                            ps = ps3.tile([P, 512], f32, tag="ps3")
                            for f in range(HL):
                                nc.tensor.matmul(
                                    ps[:], avts[f][:, tch * P:(tch + 1) * P],
                                    wps[f][:, cb_ * 512:(cb_ + 1) * 512],
                                    start=(f == 0), stop=(f == HL - 1))
                            if cb_ % 2 == 0:
                                nc.scalar.copy(
                                    out=ob[:, cb_ * 512:(cb_ + 1) * 512],
                                    in_=ps[:])
                            else:
                                nc.vector.tensor_copy(
                                    out=ob[:, cb_ * 512:(cb_ + 1) * 512],
                                    in_=ps[:])
                        if tch >= NTC - 2:
                            # split the final transfers so the drain tail is
                            # one small DMA, not a 1MB one
                            for cb_ in range(C // 512):
                                eng = nc.sync if cb_ % 2 == 0 else nc.scalar
                                eng.dma_start(
                                    out_d[tch * P:(tch + 1) * P,
                                          cb_ * 512:(cb_ + 1) * 512],
                                    ob[:, cb_ * 512:(cb_ + 1) * 512])
                        else:
                            eng = nc.sync if tch % 2 == 0 else nc.scalar
                            eng.dma_start(
                                out_d[tch * P:(tch + 1) * P, :], ob[:])
    nc.compile()
    return nc


def _make_masks():
    pp = np.arange(P)[:, None]
    ff = np.arange(P)[None, :]
    return np.where(ff >= pp, 0.0, NEG).astype(np.float32)


def _prep_inputs(x, w_qkv, w_proj):
    masks = _make_masks()
    per_g = {}
    for g in range(2):
        cols = slice(g * FL, (g + 1) * FL)
        wqk_c = np.concatenate(
            [w_qkv[:, :C][:, cols], w_qkv[:, C:2 * C][:, cols]], axis=1)
        # [jc, p, cc, col]: one contiguous DMA per jc
        wqk_packed = np.ascontiguousarray(
            wqk_c.reshape(NCC, P, 16, P).transpose(2, 1, 0, 3)).astype(np.float16)
        wv_c = np.ascontiguousarray(
            w_qkv[:, 2 * C:][:, cols].reshape(NCC, P, FL)).astype(np.float16)
        wp_c = np.ascontiguousarray(
            w_proj[g * FL:(g + 1) * FL, :]).astype(np.float16)
        per_g[g] = (wqk_packed, wv_c, wp_c)
    in_maps = []
    for core in range(NCORES):
        b, g = core // 2, core % 2
        wqk_packed, wv_c, wp_c = per_g[g]
        in_maps.append({
            "xt": np.ascontiguousarray(x[b].T).astype(np.float16),
            "wqk": wqk_packed,
            "wv": wv_c,
            "wp": wp_c,
            "masks": masks,
            "ones": np.ones((P, P), dtype=np.float16),
        })
    return in_maps


_nc_cache = None
last_results = None  # BassKernelResults of the most recent run (for test.py)


def kernel(x, w_qkv, w_proj):
    global _nc_cache, last_results
    from concourse.bass_utils import run_bass_kernel_spmd

    x = np.asarray(x, dtype=np.float32)
    w_qkv = np.asarray(w_qkv, dtype=np.float32)
    w_proj = np.asarray(w_proj, dtype=np.float32)

    if _nc_cache is None:
        _nc_cache = build_nc()
    nc = _nc_cache

    in_maps = _prep_inputs(x, w_qkv, w_proj)
    trace = bool(int(os.environ.get("KERNEL_TRACE", "0")))
    res = run_bass_kernel_spmd(nc, in_maps, list(range(NCORES)), trace=trace)
    last_results = res

    out = np.empty((B, T, C), dtype=np.float32)
    for b in range(B):
        out[b] = (res.results[2 * b]["out"].astype(np.float32)
                  + res.results[2 * b + 1]["out"].astype(np.float32))
    return out


# revision 25
# speedup vs baseline: 1.1844x; 1.1844x over previous
"""Causal self-attention Trainium2 Bass kernel (fp16 SBUF-resident rewrite).

Problem: B=4, T=2048, C=2048, H=16 heads, D=128 head dim, fp32 in/out.
  qkv = x @ w_qkv ; causal softmax(q k^T / sqrt(D)) v ; out = av @ w_proj

Sharding (8 NeuronCores): DP=4 over batch x TP=2 over head groups
(Megatron-style: w_qkv columns / w_proj rows split by heads). Core
c handles batch b=c//2, heads g*8..g*8+8 with g=c%2. Each core emits a
partial [T, C] f32 output; host sums the TP pairs.

Key differences vs the fp32r/DRAM-scratch version:
  * All operands stored fp16 (inputs cast host-side); matmuls run fp16
    at the same 1 cycle/row PE rate as f32r but with half the SBUF/DMA
    traffic, so q/k/v/av stay SBUF-resident end to end (no DRAM round
    trip between phases).
  * Softmax denominators: exp tiles accumulate on DVE (fp16 4x mode,
    lagged behind the exps so they never gate them); the partition
    reduction is one tiny [128,1]-stationary PE matmul per (head, si)
    and the broadcast of 1/den rides the otherwise-idle gpsimd.
  * Exp runs in [128,1024] two-PSUM-bank mega tiles (halves the ACT
    per-instruction access overhead); a diagonal pair left-aligns its
    second block so the pair is one contiguous exp with no garbage.
  * Phase 2 is one globally software-pipelined stream of block-pairs
    (scores two pairs ahead of AV); short diagonal-heavy si groups are
    woven into the long clean ones so the pipeline never runs shallow.
  * Phase-3 partials are written f16 (summed in f32 on host) and the
    wp loads are drip-fed on the pool queue mid-phase-2 because DMAs
    in flight across a pool transition stall its semaphore rundown.
"""

import math
import os
import sys
from collections import deque

import numpy as np

for _p in ("/opt/trn_rl_repo",):
    if _p not in sys.path:
        sys.path.insert(0, _p)

import concourse.mybir as mybir
from concourse import bacc
from concourse.tile import TileContext

B, T, C, H, D = 4, 2048, 2048, 16, 128
P = 128
NCORES = 8
HL = 8          # heads per core (local)
FL = HL * D     # local feature dim = 1024
NCC = C // P    # 16 contraction chunks
NTB = T // 512  # 4 query superblocks
NTC = T // P    # 16 t chunks
EXP_SCALE = 1.0 / math.sqrt(D)
NEG = -1.0e30

f32 = mybir.dt.float32
f16 = mybir.dt.float16


def build_nc():
    nc = bacc.Bacc()
    xt_d = nc.declare_dram_parameter("xt", [C, T], f16, isOutput=False)
    # wqk packed host-side as [jc, p, cc, col]: jc 0..7 = q head jc,
    # jc 8..15 = k head jc-8; one contiguous [128, 16*128] DMA per jc.
    wqk_d = nc.declare_dram_parameter("wqk", [16, P, NCC, P], f16, isOutput=False)
    wv_d = nc.declare_dram_parameter("wv", [NCC, P, FL], f16, isOutput=False)
    wp_d = nc.declare_dram_parameter("wp", [FL, C], f16, isOutput=False)
    masks_d = nc.declare_dram_parameter("masks", [P, P], f32, isOutput=False)
    ones_d = nc.declare_dram_parameter("ones", [P, P], f16, isOutput=False)
    out_d = nc.declare_dram_parameter("out", [T, C], f16, isOutput=True)

    ACT = mybir.ActivationFunctionType

    with TileContext(nc) as tc:
        with tc.tile_pool(name="const", bufs=1) as const_pool, \
             tc.tile_pool(name="qkt", bufs=1) as qkt_pool, \
             tc.tile_pool(name="vv", bufs=1) as v_pool:
            mask_sb = const_pool.tile([P, P], f32)
            nc.gpsimd.dma_start(mask_sb[:], masks_d[:])
            ones_sb = const_pool.tile([P, P], f16)
            nc.gpsimd.dma_start(ones_sb[:], ones_d[:])

            qkts = [qkt_pool.tile([P, T], f16, tag=f"qkt{j}", name=f"qkt{j}")
                    for j in range(16)]
            v_all = v_pool.tile([P, NTC, FL], f16, tag="vall")

            # ---------------- Phase 1: QKV projection ----------------
            with tc.tile_pool(name="xtp", bufs=1) as xt_pool:
                xts = [xt_pool.tile([P, T], f16, tag=f"xt{cc}", name=f"xt{cc}")
                       for cc in range(NCC)]

                with tc.tile_pool(name="ps1", bufs=8, space="PSUM") as ps1, \
                     tc.tile_pool(name="wv1", bufs=1) as wv_pool:
                    wvts = []
                    with tc.tile_pool(name="w1", bufs=3) as w_pool:
                        # q0/k0 weight chunks first so head 0's attention
                        # inputs exist as early as possible (jc 0 = q head 0,
                        # jc 8 = k head 0).
                        w01 = []
                        for jc in (0, 8):
                            wt_ = w_pool.tile([P, NCC, P], f16, tag="wqk",
                                              name=f"wqk{jc}")
                            nc.scalar.dma_start(wt_[:, 0:4, :], wqk_d[jc, :, 0:4, :])
                            nc.scalar.dma_start(wt_[:, 4:NCC, :],
                                                wqk_d[jc, :, 4:NCC, :])
                            w01.append(wt_)
                        # first chunk split in halves for a faster first
                        # matmul; remaining chunks rotate over three queues
                        nc.sync.dma_start(xts[0][:, 0:1024], xt_d[0:P, 0:1024])
                        nc.sync.dma_start(xts[0][:, 1024:T],
                                          xt_d[0:P, 1024:T])
                        for cc in range(1, NCC):
                            eng = nc.sync if cc % 2 == 0 else nc.scalar
                            eng.dma_start(
                                xts[cc][:], xt_d[cc * P:(cc + 1) * P, :])

                        # jc 0/8 emitted cc-outer: the PE's in-order queue
                        # then tracks xt chunk arrivals instead of blocking
                        # on the full xt load.
                        ps01 = [ps1.tile([P, 512], f32, tag="ps1",
                                         name=f"ps01_{g}") for g in range(8)]
                        for cc in range(NCC):
                            for wi in range(2):
                                for tb in range(NTB):
                                    nc.tensor.matmul(
                                        ps01[wi * NTB + tb][:],
                                        w01[wi][:, cc, :],
                                        xts[cc][:, tb * 512:(tb + 1) * 512],
                                        start=(cc == 0), stop=(cc == NCC - 1))
                        for wi, jc in enumerate((0, 8)):
                            for tb in range(NTB):
                                nc.scalar.copy(
                                    out=qkts[jc][:, tb * 512:(tb + 1) * 512],
                                    in_=ps01[wi * NTB + tb][:])

                        for jc in [j for j in range(16) if j not in (0, 8)]:
                            wt_ = w_pool.tile([P, NCC, P], f16, tag="wqk")
                            nc.scalar.dma_start(wt_[:], wqk_d[jc])
                            # cc-outer / tb-inner: 4 consecutive matmuls share
                            # the same stationary tile
                            pss = [ps1.tile([P, 512], f32, tag="ps1",
                                            name=f"psj{jc}_{tb}")
                                   for tb in range(NTB)]
                            for cc in range(NCC):
                                for tb in range(NTB):
                                    nc.tensor.matmul(
                                        pss[tb][:], wt_[:, cc, :],
                                        xts[cc][:, tb * 512:(tb + 1) * 512],
                                        start=(cc == 0), stop=(cc == NCC - 1))
                            for tb in range(NTB):
                                nc.scalar.copy(
                                    out=qkts[jc][:, tb * 512:(tb + 1) * 512],
                                    in_=pss[tb][:])

                        # wv loads stream behind the wqk stream during P1a so
                        # phase 1b starts without a handoff stall
                        for cc in range(NCC):
                            wt_ = wv_pool.tile([P, FL], f16, tag=f"wv{cc}",
                                               name=f"wv{cc}")
                            nc.scalar.dma_start(wt_[:], wv_d[cc])
                            wvts.append(wt_)

                    # 1b: v in natural [t, d] layout
                    if True:
                        for vb in range(FL // 512):
                            for tch in range(NTC):
                                ps = ps1.tile([P, 512], f32, tag="ps1")
                                for cc in range(NCC):
                                    nc.tensor.matmul(
                                        ps[:],
                                        xts[cc][:, tch * P:(tch + 1) * P],
                                        wvts[cc][:, vb * 512:(vb + 1) * 512],
                                        start=(cc == 0), stop=(cc == NCC - 1))
                                nc.scalar.copy(
                                    out=v_all[:, tch, vb * 512:(vb + 1) * 512],
                                    in_=ps[:])

            # ---------------- Phase 2: attention ----------------
            with tc.tile_pool(name="avt", bufs=1) as avt_pool, \
                 tc.tile_pool(name="wpp", bufs=1) as wp_pool:
                avts = [avt_pool.tile([P, T], f16, tag=f"avt{h}", name=f"avt{h}")
                        for h in range(HL)]
                wps = [wp_pool.tile([P, C], f16, tag=f"wp{f}", name=f"wp{f}")
                       for f in range(HL)]

                with tc.tile_pool(name="p2sb", bufs=1) as p2sb, \
                     tc.tile_pool(name="ps2", bufs=1, space="PSUM") as ps2:

                    def si_group(h, si, qt, kt):
                        njc = 4 * si + 4
                        av_ps = ps2.tile([P, 512], f32, tag="av",
                                         bufs=2, name=f"av_{h}_{si}")
                        sumacc = p2sb.tile([P, 512], f16, tag="sum",
                                           bufs=4)
                        return [(h, si, pp, njc, qt, kt, av_ps, sumacc)
                                for pp in range(njc // 2)]

                    def weave(a, b):
                        """Spread the short group b inside the long group a
                        so the pair pipeline never runs shallow and diagonal
                        (masked, partial-width) pairs are kept apart."""
                        out, bi = [], 0
                        na, nb = len(a), len(b)
                        for i, r in enumerate(a):
                            out.append(r)
                            while bi < nb and (bi + 1) * na <= (i + 1) * (nb + 1):
                                out.append(b[bi])
                                bi += 1
                        out.extend(b[bi:])
                        return out

                    # wp loads ride the pool DMA queue well after the phase
                    # boundary (in-flight DMAs crossing the transition stall
                    # its semaphore rundown); drained one per AV pair during
                    # head 1 so no single burst delays the broadcasts
                    wpq = [f for f in range(HL)]

                    def gen_pairs():
                        """Yield per-pair emission records across all
                        (head, si); scores run LOOK pairs ahead of AV."""
                        for h in range(HL):
                            qt, kt = qkts[h], qkts[8 + h]
                            recs = weave(si_group(h, 3, qt, kt),
                                         si_group(h, 0, qt, kt))
                            recs += weave(si_group(h, 2, qt, kt),
                                          si_group(h, 1, qt, kt))
                            yield from recs

                    def emit_sc(rec):
                        h, si, pp, njc, qt, kt, av_ps, sumacc = rec
                        if si == 0 and pp == 0:
                            nc.vector.memset(sumacc[:], 0)
                        sc = ps2.tile([P, 1024], f32, tag="sc", bufs=3)
                        et = p2sb.tile([P, 1024], f16, tag="et", bufs=6)
                        infos = []
                        for jj in (0, 1):
                            j = 2 * pp + jj
                            diag = j * P - si * 512
                            d_off = max(0, diag)
                            # left-align the second diagonal block so the
                            # pair's valid region is contiguous -> one exp
                            base = jj * 512 - (d_off if jj else 0)
                            nc.tensor.matmul(
                                sc[:, base + d_off:base + 512],
                                kt[:, j * P:(j + 1) * P],
                                qt[:, si * 512 + d_off:(si + 1) * 512],
                                start=True, stop=True)
                            # mask immediately after its score matmul so exp
                            # is never queued behind lower-priority DVE work
                            if diag >= 0:
                                nc.vector.tensor_add(
                                    out=sc[:, base + d_off:base + d_off + P],
                                    in0=sc[:, base + d_off:base + d_off + P],
                                    in1=mask_sb[:])
                            infos.append((j, diag >= 0, d_off, base))
                        lo = infos[0][2] + infos[0][3]  # d_off0 (+ base0 == 0)
                        hi = infos[1][3] + 512          # base1 + 512
                        nc.scalar.activation(et[:, lo:hi], sc[:, lo:hi],
                                             ACT.Exp, scale=EXP_SCALE)
                        return (rec, et, infos)

                    def emit_av(sc_rec):
                        (h, si, pp, njc, qt, kt, av_ps, sumacc), et, infos = sc_rec
                        last_pair = (2 * pp + 1 == njc - 1)
                        if h >= 1 and wpq:
                            f = wpq.pop(0)
                            nc.gpsimd.dma_start(
                                wps[f][:], wp_d[f * P:(f + 1) * P, :])
                        for (j, is_diag, d_off, base) in infos:
                            nc.tensor.matmul(
                                av_ps[:, d_off:],
                                v_all[:, j, h * P:(h + 1) * P],
                                et[:, base + d_off:base + 512],
                                start=(j == 0), stop=(j == njc - 1))
                        if not last_pair:
                            # denominator accumulation lags the exps by LOOK
                            # pairs: it only gates the si-end ones-matmul,
                            # never the next exp. First pair initializes
                            # sumacc without a memset (except si 0, whose
                            # first pair is diagonal/partial-width).
                            if pp == 0 and si > 0:
                                nc.vector.scalar_tensor_tensor(
                                    out=sumacc[:], in0=et[:, 0:512],
                                    scalar=1.0, in1=et[:, 512:1024],
                                    op0=mybir.AluOpType.mult,
                                    op1=mybir.AluOpType.add)
                            else:
                                for (j, is_diag, d_off, base) in infos:
                                    nc.vector.tensor_add(
                                        out=sumacc[:, d_off:],
                                        in0=sumacc[:, d_off:],
                                        in1=et[:, base + d_off:base + 512])
                        else:
                            # close out softmax. The last pair's exps are
                            # summed directly by the PE ones-matmul (PSUM
                            # accumulation), so the chain never waits on a
                            # trailing DVE sumadd; broadcast rides gpsimd
                            # off the critical path.
                            den = ps2.tile([P, 1024], f32, tag="sc", bufs=3,
                                           name=f"den_{h}_{si}")
                            nc.tensor.matmul(
                                den[0:1, 0:512], ones_sb[:, 0:1], sumacc[:],
                                start=True, stop=False)
                            for (j, is_diag, d_off, base) in infos:
                                nc.tensor.matmul(
                                    den[0:1, d_off:512], ones_sb[:, 0:1],
                                    et[:, base + d_off:base + 512],
                                    start=False, stop=(j == njc - 1))
                            rec32 = p2sb.tile([1, 512], f32, tag="rec32",
                                              bufs=3)
                            nc.vector.reciprocal_approx_fast(
                                out=rec32[:], in_=den[0:1, 0:512])
                            recb = p2sb.tile([P, 512], f32, tag="recb",
                                             bufs=3)
                            nc.gpsimd.partition_broadcast(recb[:], rec32[:])
                            nc.vector.tensor_mul(
                                out=avts[h][:, si * 512:(si + 1) * 512],
                                in0=av_ps[:], in1=recb[:])

                    LOOK = 2
                    pend = deque()
                    for rec in gen_pairs():
                        pend.append(emit_sc(rec))
                        if len(pend) > LOOK:
                            emit_av(pend.popleft())
                    while pend:
                        emit_av(pend.popleft())

                # ---------------- Phase 3: output projection ----------------
                with tc.tile_pool(name="cb3", bufs=6) as cb3, \
                     tc.tile_pool(name="ps3", bufs=6, space="PSUM") as ps3:
                    for tch in range(NTC):
                        ob = cb3.tile([P, C], f16, tag="ocb")
                        for cb_ in range(C // 512):
                            ps = ps3.tile([P, 512], f32, tag="ps3")
                            for f in range(HL):
                                nc.tensor.matmul(
                                    ps[:], avts[f][:, tch * P:(tch + 1) * P],
                                    wps[f][:, cb_ * 512:(cb_ + 1) * 512],
                                    start=(f == 0), stop=(f == HL - 1))
                            if cb_ % 2 == 0:
                                nc.scalar.copy(
                                    out=ob[:, cb_ * 512:(cb_ + 1) * 512],
                                    in_=ps[:])
                            else:
                                nc.vector.tensor_copy(
                                    out=ob[:, cb_ * 512:(cb_ + 1) * 512],
                                    in_=ps[:])
                        if tch >= NTC - 2:
                            # split the final transfers so the drain tail is
                            # one small DMA, not a 1MB one
                            for cb_ in range(C // 512):
                                eng = nc.sync if cb_ % 2 == 0 else nc.scalar
                                eng.dma_start(
                                    out_d[tch * P:(tch + 1) * P,
                                          cb_ * 512:(cb_ + 1) * 512],
                                    ob[:, cb_ * 512:(cb_ + 1) * 512])
                        else:
                            eng = nc.sync if tch % 2 == 0 else nc.scalar
                            eng.dma_start(
                                out_d[tch * P:(tch + 1) * P, :], ob[:])
    nc.compile()
    return nc


def _make_masks():
    pp = np.arange(P)[:, None]
    ff = np.arange(P)[None, :]
    return np.where(ff >= pp, 0.0, NEG).astype(np.float32)


def _prep_inputs(x, w_qkv, w_proj):
    masks = _make_masks()
    per_g = {}
    for g in range(2):
        cols = slice(g * FL, (g + 1) * FL)
        wqk_c = np.concatenate(
            [w_qkv[:, :C][:, cols], w_qkv[:, C:2 * C][:, cols]], axis=1)
        # [jc, p, cc, col]: one contiguous DMA per jc
        wqk_packed = np.ascontiguousarray(
            wqk_c.reshape(NCC, P, 16, P).transpose(2, 1, 0, 3)).astype(np.float16)
        wv_c = np.ascontiguousarray(
            w_qkv[:, 2 * C:][:, cols].reshape(NCC, P, FL)).astype(np.float16)
        wp_c = np.ascontiguousarray(
            w_proj[g * FL:(g + 1) * FL, :]).astype(np.float16)
        per_g[g] = (wqk_packed, wv_c, wp_c)
    in_maps = []
    for core in range(NCORES):
        b, g = core // 2, core % 2
        wqk_packed, wv_c, wp_c = per_g[g]
        in_maps.append({
            "xt": np.ascontiguousarray(x[b].T).astype(np.float16),
            "wqk": wqk_packed,
            "wv": wv_c,
            "wp": wp_c,
            "masks": masks,
            "ones": np.ones((P, P), dtype=np.float16),
        })
    return in_maps


_nc_cache = None
last_results = None  # BassKernelResults of the most recent run (for test.py)


def kernel(x, w_qkv, w_proj):
    global _nc_cache, last_results
    from concourse.bass_utils import run_bass_kernel_spmd

    x = np.asarray(x, dtype=np.float32)
    w_qkv = np.asarray(w_qkv, dtype=np.float32)
    w_proj = np.asarray(w_proj, dtype=np.float32)

    if _nc_cache is None:
        _nc_cache = build_nc()
    nc = _nc_cache

    in_maps = _prep_inputs(x, w_qkv, w_proj)
    trace = bool(int(os.environ.get("KERNEL_TRACE", "0")))
    res = run_bass_kernel_spmd(nc, in_maps, list(range(NCORES)), trace=trace)
    last_results = res

    out = np.empty((B, T, C), dtype=np.float32)
    for b in range(B):
        out[b] = (res.results[2 * b]["out"].astype(np.float32)
                  + res.results[2 * b + 1]["out"].astype(np.float32))
    return out


# revision 26
# speedup vs baseline: 1.1865x; 1.0018x over previous
"""Causal self-attention Trainium2 Bass kernel (fp16 SBUF-resident rewrite).

Problem: B=4, T=2048, C=2048, H=16 heads, D=128 head dim, fp32 in/out.
  qkv = x @ w_qkv ; causal softmax(q k^T / sqrt(D)) v ; out = av @ w_proj

Sharding (8 NeuronCores): DP=4 over batch x TP=2 over head groups
(Megatron-style: w_qkv columns / w_proj rows split by heads). Core
c handles batch b=c//2, heads g*8..g*8+8 with g=c%2. Each core emits a
partial [T, C] f32 output; host sums the TP pairs.

Key differences vs the fp32r/DRAM-scratch version:
  * All operands stored fp16 (inputs cast host-side); matmuls run fp16
    at the same 1 cycle/row PE rate as f32r but with half the SBUF/DMA
    traffic, so q/k/v/av stay SBUF-resident end to end (no DRAM round
    trip between phases).
  * Softmax denominators: exp tiles accumulate on DVE (fp16 4x mode,
    lagged behind the exps so they never gate them); the partition
    reduction is one tiny [128,1]-stationary PE matmul per (head, si)
    and the broadcast of 1/den rides the otherwise-idle gpsimd.
  * Exp runs in [128,1024] two-PSUM-bank mega tiles (halves the ACT
    per-instruction access overhead); a diagonal pair left-aligns its
    second block so the pair is one contiguous exp with no garbage.
  * Phase 2 is one globally software-pipelined stream of block-pairs
    (scores two pairs ahead of AV); short diagonal-heavy si groups are
    woven into the long clean ones so the pipeline never runs shallow.
  * Phase-3 partials are written f16 (summed in f32 on host) and the
    wp loads are drip-fed on the pool queue mid-phase-2 because DMAs
    in flight across a pool transition stall its semaphore rundown.
"""

import math
import os
import sys
from collections import deque

import numpy as np

for _p in ("/opt/trn_rl_repo",):
    if _p not in sys.path:
        sys.path.insert(0, _p)

import concourse.mybir as mybir
from concourse import bacc
from concourse.tile import TileContext

B, T, C, H, D = 4, 2048, 2048, 16, 128
P = 128
NCORES = 8
HL = 8          # heads per core (local)
FL = HL * D     # local feature dim = 1024
NCC = C // P    # 16 contraction chunks
NTB = T // 512  # 4 query superblocks
NTC = T // P    # 16 t chunks
EXP_SCALE = 1.0 / math.sqrt(D)
NEG = -1.0e30

f32 = mybir.dt.float32
f16 = mybir.dt.float16


def build_nc():
    nc = bacc.Bacc()
    xt_d = nc.declare_dram_parameter("xt", [C, T], f16, isOutput=False)
    # wqk packed host-side as [jc, p, cc, col]: jc 0..7 = q head jc,
    # jc 8..15 = k head jc-8; one contiguous [128, 16*128] DMA per jc.
    wqk_d = nc.declare_dram_parameter("wqk", [16, P, NCC, P], f16, isOutput=False)
    wv_d = nc.declare_dram_parameter("wv", [NCC, P, FL], f16, isOutput=False)
    wp_d = nc.declare_dram_parameter("wp", [FL, C], f16, isOutput=False)
    masks_d = nc.declare_dram_parameter("masks", [P, P], f32, isOutput=False)
    ones_d = nc.declare_dram_parameter("ones", [P, P], f16, isOutput=False)
    out_d = nc.declare_dram_parameter("out", [T, C], f16, isOutput=True)

    ACT = mybir.ActivationFunctionType

    with TileContext(nc) as tc:
        with tc.tile_pool(name="const", bufs=1) as const_pool, \
             tc.tile_pool(name="qkt", bufs=1) as qkt_pool, \
             tc.tile_pool(name="vv", bufs=1) as v_pool:
            mask_sb = const_pool.tile([P, P], f32)
            nc.gpsimd.dma_start(mask_sb[:], masks_d[:])
            ones_sb = const_pool.tile([P, P], f16)
            nc.gpsimd.dma_start(ones_sb[:], ones_d[:])

            qkts = [qkt_pool.tile([P, T], f16, tag=f"qkt{j}", name=f"qkt{j}")
                    for j in range(16)]
            v_all = v_pool.tile([P, NTC, FL], f16, tag="vall")

            # ---------------- Phase 1: QKV projection ----------------
            with tc.tile_pool(name="xtp", bufs=1) as xt_pool:
                xts = [xt_pool.tile([P, T], f16, tag=f"xt{cc}", name=f"xt{cc}")
                       for cc in range(NCC)]

                with tc.tile_pool(name="ps1", bufs=8, space="PSUM") as ps1, \
                     tc.tile_pool(name="wv1", bufs=1) as wv_pool:
                    wvts = []
                    with tc.tile_pool(name="w1", bufs=3) as w_pool:
                        # q0/k0 weight chunks first so head 0's attention
                        # inputs exist as early as possible (jc 0 = q head 0,
                        # jc 8 = k head 0).
                        w01 = []
                        for jc in (0, 8):
                            wt_ = w_pool.tile([P, NCC, P], f16, tag="wqk",
                                              name=f"wqk{jc}")
                            w01.append(wt_)
                        # scalar queue interleave: the halves covering cc 0-3
                        # first (all early matmuls need them), then xt1, then
                        # the cc 4-15 halves, so both input streams flow from
                        # t=0 and the PE's chunk consumption is never starved
                        nc.scalar.dma_start(w01[0][:, 0:4, :], wqk_d[0, :, 0:4, :])
                        nc.scalar.dma_start(w01[1][:, 0:4, :], wqk_d[8, :, 0:4, :])
                        nc.sync.dma_start(xts[0][:, 0:1024], xt_d[0:P, 0:1024])
                        nc.sync.dma_start(xts[0][:, 1024:T],
                                          xt_d[0:P, 1024:T])
                        nc.scalar.dma_start(xts[1][:], xt_d[P:2 * P, :])
                        nc.scalar.dma_start(w01[0][:, 4:NCC, :],
                                            wqk_d[0, :, 4:NCC, :])
                        nc.scalar.dma_start(w01[1][:, 4:NCC, :],
                                            wqk_d[8, :, 4:NCC, :])
                        for cc in range(2, NCC):
                            eng = nc.sync if cc % 2 == 0 else nc.scalar
                            eng.dma_start(
                                xts[cc][:], xt_d[cc * P:(cc + 1) * P, :])

                        # jc 0/8 emitted cc-outer: the PE's in-order queue
                        # then tracks xt chunk arrivals instead of blocking
                        # on the full xt load.
                        ps01 = [ps1.tile([P, 512], f32, tag="ps1",
                                         name=f"ps01_{g}") for g in range(8)]
                        for cc in range(NCC):
                            for wi in range(2):
                                for tb in range(NTB):
                                    nc.tensor.matmul(
                                        ps01[wi * NTB + tb][:],
                                        w01[wi][:, cc, :],
                                        xts[cc][:, tb * 512:(tb + 1) * 512],
                                        start=(cc == 0), stop=(cc == NCC - 1))
                        for wi, jc in enumerate((0, 8)):
                            for tb in range(NTB):
                                nc.scalar.copy(
                                    out=qkts[jc][:, tb * 512:(tb + 1) * 512],
                                    in_=ps01[wi * NTB + tb][:])

                        for jc in [j for j in range(16) if j not in (0, 8)]:
                            wt_ = w_pool.tile([P, NCC, P], f16, tag="wqk")
                            nc.scalar.dma_start(wt_[:], wqk_d[jc])
                            # cc-outer / tb-inner: 4 consecutive matmuls share
                            # the same stationary tile
                            pss = [ps1.tile([P, 512], f32, tag="ps1",
                                            name=f"psj{jc}_{tb}")
                                   for tb in range(NTB)]
                            for cc in range(NCC):
                                for tb in range(NTB):
                                    nc.tensor.matmul(
                                        pss[tb][:], wt_[:, cc, :],
                                        xts[cc][:, tb * 512:(tb + 1) * 512],
                                        start=(cc == 0), stop=(cc == NCC - 1))
                            for tb in range(NTB):
                                nc.scalar.copy(
                                    out=qkts[jc][:, tb * 512:(tb + 1) * 512],
                                    in_=pss[tb][:])

                        # wv loads stream behind the wqk stream during P1a so
                        # phase 1b starts without a handoff stall
                        for cc in range(NCC):
                            wt_ = wv_pool.tile([P, FL], f16, tag=f"wv{cc}",
                                               name=f"wv{cc}")
                            nc.scalar.dma_start(wt_[:], wv_d[cc])
                            wvts.append(wt_)

                    # 1b: v in natural [t, d] layout
                    if True:
                        for vb in range(FL // 512):
                            for tch in range(NTC):
                                ps = ps1.tile([P, 512], f32, tag="ps1")
                                for cc in range(NCC):
                                    nc.tensor.matmul(
                                        ps[:],
                                        xts[cc][:, tch * P:(tch + 1) * P],
                                        wvts[cc][:, vb * 512:(vb + 1) * 512],
                                        start=(cc == 0), stop=(cc == NCC - 1))
                                nc.vector.tensor_copy(
                                    out=v_all[:, tch, vb * 512:(vb + 1) * 512],
                                    in_=ps[:])

            # ---------------- Phase 2: attention ----------------
            with tc.tile_pool(name="avt", bufs=1) as avt_pool, \
                 tc.tile_pool(name="wpp", bufs=1) as wp_pool:
                avts = [avt_pool.tile([P, T], f16, tag=f"avt{h}", name=f"avt{h}")
                        for h in range(HL)]
                wps = [wp_pool.tile([P, C], f16, tag=f"wp{f}", name=f"wp{f}")
                       for f in range(HL)]

                with tc.tile_pool(name="p2sb", bufs=1) as p2sb, \
                     tc.tile_pool(name="ps2", bufs=1, space="PSUM") as ps2:

                    def si_group(h, si, qt, kt):
                        njc = 4 * si + 4
                        av_ps = ps2.tile([P, 512], f32, tag="av",
                                         bufs=2, name=f"av_{h}_{si}")
                        sumacc = p2sb.tile([P, 512], f16, tag="sum",
                                           bufs=4)
                        return [(h, si, pp, njc, qt, kt, av_ps, sumacc)
                                for pp in range(njc // 2)]

                    def weave(a, b):
                        """Spread the short group b inside the long group a
                        so the pair pipeline never runs shallow and diagonal
                        (masked, partial-width) pairs are kept apart."""
                        out, bi = [], 0
                        na, nb = len(a), len(b)
                        for i, r in enumerate(a):
                            out.append(r)
                            while bi < nb and (bi + 1) * na <= (i + 1) * (nb + 1):
                                out.append(b[bi])
                                bi += 1
                        out.extend(b[bi:])
                        return out

                    # wp loads ride the pool DMA queue well after the phase
                    # boundary (in-flight DMAs crossing the transition stall
                    # its semaphore rundown); drained one per AV pair during
                    # head 1 so no single burst delays the broadcasts
                    wpq = [f for f in range(HL)]

                    def gen_pairs():
                        """Yield per-pair emission records across all
                        (head, si); scores run LOOK pairs ahead of AV."""
                        for h in range(HL):
                            qt, kt = qkts[h], qkts[8 + h]
                            recs = weave(si_group(h, 3, qt, kt),
                                         si_group(h, 0, qt, kt))
                            recs += weave(si_group(h, 2, qt, kt),
                                          si_group(h, 1, qt, kt))
                            yield from recs

                    def emit_sc(rec):
                        h, si, pp, njc, qt, kt, av_ps, sumacc = rec
                        if si == 0 and pp == 0:
                            nc.vector.memset(sumacc[:], 0)
                        sc = ps2.tile([P, 1024], f32, tag="sc", bufs=3)
                        et = p2sb.tile([P, 1024], f16, tag="et", bufs=6)
                        infos = []
                        for jj in (0, 1):
                            j = 2 * pp + jj
                            diag = j * P - si * 512
                            d_off = max(0, diag)
                            # left-align the second diagonal block so the
                            # pair's valid region is contiguous -> one exp
                            base = jj * 512 - (d_off if jj else 0)
                            nc.tensor.matmul(
                                sc[:, base + d_off:base + 512],
                                kt[:, j * P:(j + 1) * P],
                                qt[:, si * 512 + d_off:(si + 1) * 512],
                                start=True, stop=True)
                            # mask immediately after its score matmul so exp
                            # is never queued behind lower-priority DVE work
                            if diag >= 0:
                                nc.vector.tensor_add(
                                    out=sc[:, base + d_off:base + d_off + P],
                                    in0=sc[:, base + d_off:base + d_off + P],
                                    in1=mask_sb[:])
                            infos.append((j, diag >= 0, d_off, base))
                        lo = infos[0][2] + infos[0][3]  # d_off0 (+ base0 == 0)
                        hi = infos[1][3] + 512          # base1 + 512
                        nc.scalar.activation(et[:, lo:hi], sc[:, lo:hi],
                                             ACT.Exp, scale=EXP_SCALE)
                        return (rec, et, infos)

                    def emit_av(sc_rec):
                        (h, si, pp, njc, qt, kt, av_ps, sumacc), et, infos = sc_rec
                        last_pair = (2 * pp + 1 == njc - 1)
                        if h >= 1 and wpq:
                            f = wpq.pop(0)
                            nc.gpsimd.dma_start(
                                wps[f][:], wp_d[f * P:(f + 1) * P, :])
                        for (j, is_diag, d_off, base) in infos:
                            nc.tensor.matmul(
                                av_ps[:, d_off:],
                                v_all[:, j, h * P:(h + 1) * P],
                                et[:, base + d_off:base + 512],
                                start=(j == 0), stop=(j == njc - 1))
                        if not last_pair:
                            # denominator accumulation lags the exps by LOOK
                            # pairs: it only gates the si-end ones-matmul,
                            # never the next exp. First pair initializes
                            # sumacc without a memset (except si 0, whose
                            # first pair is diagonal/partial-width).
                            if pp == 0 and si > 0:
                                nc.vector.scalar_tensor_tensor(
                                    out=sumacc[:], in0=et[:, 0:512],
                                    scalar=1.0, in1=et[:, 512:1024],
                                    op0=mybir.AluOpType.mult,
                                    op1=mybir.AluOpType.add)
                            else:
                                for (j, is_diag, d_off, base) in infos:
                                    nc.vector.tensor_add(
                                        out=sumacc[:, d_off:],
                                        in0=sumacc[:, d_off:],
                                        in1=et[:, base + d_off:base + 512])
                        else:
                            # close out softmax. The last pair's exps are
                            # summed directly by the PE ones-matmul (PSUM
                            # accumulation), so the chain never waits on a
                            # trailing DVE sumadd; broadcast rides gpsimd
                            # off the critical path.
                            den = ps2.tile([P, 1024], f32, tag="sc", bufs=3,
                                           name=f"den_{h}_{si}")
                            nc.tensor.matmul(
                                den[0:1, 0:512], ones_sb[:, 0:1], sumacc[:],
                                start=True, stop=False)
                            for (j, is_diag, d_off, base) in infos:
                                nc.tensor.matmul(
                                    den[0:1, d_off:512], ones_sb[:, 0:1],
                                    et[:, base + d_off:base + 512],
                                    start=False, stop=(j == njc - 1))
                            rec32 = p2sb.tile([1, 512], f32, tag="rec32",
                                              bufs=3)
                            nc.vector.reciprocal_approx_fast(
                                out=rec32[:], in_=den[0:1, 0:512])
                            recb = p2sb.tile([P, 512], f32, tag="recb",
                                             bufs=3)
                            nc.gpsimd.partition_broadcast(recb[:], rec32[:])
                            nc.vector.tensor_mul(
                                out=avts[h][:, si * 512:(si + 1) * 512],
                                in0=av_ps[:], in1=recb[:])

                    LOOK = 2
                    pend = deque()
                    for rec in gen_pairs():
                        pend.append(emit_sc(rec))
                        if len(pend) > LOOK:
                            emit_av(pend.popleft())
                    while pend:
                        emit_av(pend.popleft())

                # ---------------- Phase 3: output projection ----------------
                with tc.tile_pool(name="cb3", bufs=6) as cb3, \
                     tc.tile_pool(name="ps3", bufs=6, space="PSUM") as ps3:
                    for tch in range(NTC):
                        ob = cb3.tile([P, C], f16, tag="ocb")
                        for cb_ in range(C // 512):
                            ps = ps3.tile([P, 512], f32, tag="ps3")
                            for f in range(HL):
                                nc.tensor.matmul(
                                    ps[:], avts[f][:, tch * P:(tch + 1) * P],
                                    wps[f][:, cb_ * 512:(cb_ + 1) * 512],
                                    start=(f == 0), stop=(f == HL - 1))
                            if cb_ % 2 == 0:
                                nc.scalar.copy(
                                    out=ob[:, cb_ * 512:(cb_ + 1) * 512],
                                    in_=ps[:])
                            else:
                                nc.vector.tensor_copy(
                                    out=ob[:, cb_ * 512:(cb_ + 1) * 512],
                                    in_=ps[:])
                        if tch >= NTC - 2:
                            # split the final transfers so the drain tail is
                            # one small DMA, not a 1MB one
                            for cb_ in range(C // 512):
                                eng = nc.sync if cb_ % 2 == 0 else nc.scalar
                                eng.dma_start(
                                    out_d[tch * P:(tch + 1) * P,
                                          cb_ * 512:(cb_ + 1) * 512],
                                    ob[:, cb_ * 512:(cb_ + 1) * 512])
                        else:
                            eng = nc.sync if tch % 2 == 0 else nc.scalar
                            eng.dma_start(
                                out_d[tch * P:(tch + 1) * P, :], ob[:])
    nc.compile()
    return nc


def _make_masks():
    pp = np.arange(P)[:, None]
    ff = np.arange(P)[None, :]
    return np.where(ff >= pp, 0.0, NEG).astype(np.float32)


def _prep_inputs(x, w_qkv, w_proj):
    masks = _make_masks()
    per_g = {}
    for g in range(2):
        cols = slice(g * FL, (g + 1) * FL)
        wqk_c = np.concatenate(
            [w_qkv[:, :C][:, cols], w_qkv[:, C:2 * C][:, cols]], axis=1)
        # [jc, p, cc, col]: one contiguous DMA per jc
        wqk_packed = np.ascontiguousarray(
            wqk_c.reshape(NCC, P, 16, P).transpose(2, 1, 0, 3)).astype(np.float16)
        wv_c = np.ascontiguousarray(
            w_qkv[:, 2 * C:][:, cols].reshape(NCC, P, FL)).astype(np.float16)
        wp_c = np.ascontiguousarray(
            w_proj[g * FL:(g + 1) * FL, :]).astype(np.float16)
        per_g[g] = (wqk_packed, wv_c, wp_c)
    in_maps = []
    for core in range(NCORES):
        b, g = core // 2, core % 2
        wqk_packed, wv_c, wp_c = per_g[g]
        in_maps.append({
            "xt": np.ascontiguousarray(x[b].T).astype(np.float16),
            "wqk": wqk_packed,
            "wv": wv_c,
            "wp": wp_c,
            "masks": masks,
            "ones": np.ones((P, P), dtype=np.float16),
        })
    return in_maps


_nc_cache = None
last_results = None  # BassKernelResults of the most recent run (for test.py)


def kernel(x, w_qkv, w_proj):
    global _nc_cache, last_results
    from concourse.bass_utils import run_bass_kernel_spmd

    x = np.asarray(x, dtype=np.float32)
    w_qkv = np.asarray(w_qkv, dtype=np.float32)
    w_proj = np.asarray(w_proj, dtype=np.float32)

    if _nc_cache is None:
        _nc_cache = build_nc()
    nc = _nc_cache

    in_maps = _prep_inputs(x, w_qkv, w_proj)
    trace = bool(int(os.environ.get("KERNEL_TRACE", "0")))
    res = run_bass_kernel_spmd(nc, in_maps, list(range(NCORES)), trace=trace)
    last_results = res

    out = np.empty((B, T, C), dtype=np.float32)
    for b in range(B):
        out[b] = (res.results[2 * b]["out"].astype(np.float32)
                  + res.results[2 * b + 1]["out"].astype(np.float32))
    return out
